# revision 1
# baseline (speedup 1.0000x reference)
"""MACCL loss kernel for Trainium2 (8 NeuronCores, SPMD data-parallel).

Strategy
--------
The O(B^2 D) contrastive part dominates (B=8192, D=256).  We permute the
batch so label-0 rows come first (split point n0 is baked into the
program at build time), shard rows 1024-per-core, and on each core:

  prologue (streamed in 16-row-tile chunks, overlapped with main loop):
    - DMA raw feature tiles [128, 256] fp32
    - row norms^2 via DVE tensor_tensor_reduce
    - rsqrt via ACT Ln + Exp(scale=-0.5)  (keeps ACT on ONE table set:
      natural_log_exp_and_others = {Ln, Exp, Copy, Square})
    - normalize rows on GPSIMD (tensor_scalar_mul)
    - PE transpose into a resident SBUF [128, 2, 8192] f-hat^T operand

  main loop (g-outer over 2048-wide column groups, m-inner over the
  core's eight 128-row tiles):
    - PE matmul f_hat_mine^T.T @ f_hat_all^T  (float32r: full fp32 data
      at 1 cycle/row) accumulating K=256 in two passes into PSUM
    - ACT exp(psum/T) in place with accum_out giving per-row sums per
      label segment (columns are label-sorted, so pos/neg sums are just
      contiguous column-range sums)
    - DVE reduces the few segment partials into S0/S1

  outputs per core: stats [128, 40] fp32 = {norms^2, rowsum, S0, S1,
  exp(diag)} for its 1024 rows.  Host does the O(B) finalization
  (sigma/margin/log/mean) exactly mirroring the reference formulas.

The diagonal (self-similarity) term is computed on-device from the same
normalized values the PE consumes and subtracted on the host, so no
per-core mask positions are needed (the program stays SPMD-uniform).
"""

import os
import sys

for _p in ("/root/.axon_site", "/root/.axon_site/_ro/trn_rl_repo",
           "/root/.axon_site/_ro/pypackages", "/opt/trn_rl_repo", "/opt/pypackages"):
    if os.path.isdir(_p) and _p not in sys.path:
        sys.path.append(_p)

import numpy as np
from contextlib import ExitStack

import concourse.bass as bass
import concourse.bacc as bacc
import concourse.tile as tile
from concourse import mybir
from concourse.bass_utils import run_bass_kernel_spmd

F32 = mybir.dt.float32
F32R = mybir.dt.float32r
BF16 = mybir.dt.bfloat16
F16 = mybir.dt.float16

P = 128
D = 256
NCORES = 8
TEMPERATURE = 0.07
MARGIN_BASE = 0.5
LAMBDA_SIGMA = 0.3
LAMBDA_RESOLUTION = 0.3
RESOLUTION_RATIO = 224.0 / 900.0
ALPHA, BETA, GAMMA = 1.0, 1.0, 0.5

# matmul dtype mode: "f32r" (fp32 data, 1 cyc/row), "bf16", or "f32" (4 cyc/row)
MAT_MODE = os.environ.get("MACCL_MAT_MODE", "f32r")


def _segment_ranges(B, n0, gw):
    """Column ranges per 2048-wide group, split at the label boundary n0.

    Returns (ranges, k0): ranges = [(g, start, end, label)...] in ascending
    column order (so all label-0 ranges come first), k0 = #label-0 ranges.
    """
    ranges = []
    ng = B // gw
    for g in range(ng):
        lo, hi = g * gw, (g + 1) * gw
        cuts = sorted({lo, hi, min(max(n0, lo), hi)})
        for s, e in zip(cuts, cuts[1:]):
            if e > s:
                ranges.append((g, s, e, 0 if e <= n0 else 1))
    k0 = sum(1 for r in ranges if r[3] == 0)
    return ranges, k0


def build_program(n0, B=8192, bpc=1024, mat_mode=MAT_MODE):
    """Build the SPMD Bass program (one NeuronCore's view)."""
    gw = 2048 if B % 2048 == 0 else B
    ng = B // gw
    nsub = gw // 512
    nt_all = B // P
    nt_mine = bpc // P
    mrow = bpc // P
    assert nt_mine == mrow

    use_bf16 = mat_mode == "bf16"
    # float32r is a reduced-precision fp32 variant: walrus requires matmul
    # operand producers to round to f32r, so the operand tiles are native
    # f32r and the PSUM->SBUF copies do the rounding.
    store_dt = {"bf16": BF16, "f32r": F32R, "f32": F32, "f16": F16}[mat_mode]

    ranges, k0 = _segment_ranges(B, n0, gw)
    nslots = len(ranges)
    k1 = nslots - k0

    nc = bacc.Bacc("TRN2", target_bir_lowering=False, debug=False,
                   num_devices=NCORES)
    feat_all = nc.dram_tensor("feat_all", [B, D], F32, kind="ExternalInput").ap()
    feat_mine = nc.dram_tensor("feat_mine", [bpc, D], F32, kind="ExternalInput").ap()
    ident_d = nc.dram_tensor("ident", [P, P], F32, kind="ExternalInput").ap()
    stats_d = nc.dram_tensor("stats", [P, 5 * mrow], F32, kind="ExternalOutput").ap()

    fa_r = feat_all.rearrange("(n p) d -> n p d", p=P)
    fm_r = feat_mine.rearrange("(n p) d -> n p d", p=P)

    AX = mybir.AxisListType.X
    MUL = mybir.AluOpType.mult
    ADD = mybir.AluOpType.add
    AF = mybir.ActivationFunctionType

    with tile.TileContext(nc) as tc, ExitStack() as ctx:
        singles = ctx.enter_context(tc.tile_pool(name="singles", bufs=1))
        raw_pool = ctx.enter_context(tc.tile_pool(name="raw", bufs=20))
        b_pool = ctx.enter_context(tc.tile_pool(name="bpool", bufs=5))
        scr_pool = ctx.enter_context(tc.tile_pool(name="scr", bufs=2))
        small = ctx.enter_context(tc.tile_pool(name="small", bufs=3))
        acc_pool = ctx.enter_context(tc.tile_pool(name="acc", bufs=mrow))
        ps_pool = ctx.enter_context(tc.tile_pool(name="ps", bufs=2, space="PSUM"))

        ident_t = singles.tile([P, P], F32)
        nc.sync.dma_start(ident_t, ident_d)
        allT = singles.tile([P, 2, B], store_dt)
        mineT = singles.tile([P, 2, bpc], store_dt)
        stats_sb = singles.tile([P, 5 * mrow], F32)


        def prologue_chunk(srcs, destT, col0, mine_base):
            """Process a chunk of row tiles: stats, normalize, transpose.

            srcs: list of DRAM [128, 256] APs.  destT: allT or mineT.
            col0: first destination column tile index.  mine_base: row-tile
            index of srcs[0] within the core's own block, or None.
            """
            n = len(srcs)
            nrm2c = small.tile([P, n], F32, tag="nrm2c")
            raws = []
            for i, src in enumerate(srcs):
                raw = raw_pool.tile([P, D], F32, tag="raw")
                nc.sync.dma_start(raw, src)
                scr = scr_pool.tile([P, D], F32, tag="scr")
                nc.vector.scalar_tensor_tensor(
                    out=scr, in0=raw, scalar=1.0, in1=raw,
                    op0=MUL, op1=MUL, accum_out=nrm2c[:, i:i + 1])
                if mine_base is not None:
                    mi = mine_base + i
                    nc.vector.reduce_sum(stats_sb[:, mrow + mi:mrow + mi + 1],
                                         raw, axis=AX)
                raws.append(raw)
            if mine_base is not None:
                # norms^2 for own rows (before clamping; values >> clamp).
                # Keep on DVE: a Pool reader of nrm2c adds cross-engine waits
                # that overflow the tensor-scalar sync-wait budget in walrus.
                nc.vector.tensor_copy(out=stats_sb[:, 0:n], in_=nrm2c[:, 0:n])
            nc.vector.tensor_scalar_max(nrm2c, nrm2c, 1e-24)
            lnc = small.tile([P, n], F32, tag="lnc")
            nc.scalar.activation(lnc, nrm2c, AF.Ln)
            rcpc = small.tile([P, n], F32, tag="rcpc")
            nc.scalar.activation(rcpc, lnc, AF.Exp, scale=-0.5)
            for i in range(n):
                b = b_pool.tile([P, D], F32, tag="b")
                # ACT Copy with per-partition scale: Pool tensor_scalar with
                # an AP scalar measures ~3.9us/op on HW (ucode path) and was
                # the dominant bottleneck; ACT does this in ~0.3us.
                nc.scalar.activation(b, raws[i], AF.Copy,
                                     scale=rcpc[:, i:i + 1])
                pt = ps_pool.tile([P, gw], F32, tag="ps")
                for h in (0, 1):
                    nc.tensor.transpose(pt[:, h * 512:h * 512 + P],
                                        b[:, h * P:(h + 1) * P], ident_t)
                col = (col0 + i) * P
                nc.vector.tensor_copy(out=destT[:, 0, col:col + P],
                                      in_=pt[:, 0:P])
                nc.vector.tensor_copy(out=destT[:, 1, col:col + P],
                                      in_=pt[:, 512:512 + P])

        # ---- own rows first (mineT + per-row stats) ----
        prologue_chunk([fm_r[i, :, :] for i in range(nt_mine)], mineT, 0, 0)

        # ---- exact diagonal terms ----
        # psd = mineT_m^T @ mineT_m reproduces, bit-for-bit, the diagonal
        # elements the big matmul produces (same stationary column, same
        # moving column, same f32r datapath); exp through the same ACT path
        # then a masked row-reduce against the identity extracts e_ii, so
        # the host-side pos_sum = S_same - d subtraction cancels exactly.
        for m in range(mrow):
            psd = ps_pool.tile([P, gw], F32, tag="ps", name=f"psd{m}")
            W = min(512, bpc)                  # matches main-matmul N on full cfg
            c0 = (m * P // W) * W              # W-col group holding block m
            off = m * P - c0                   # block-local diag offset
            for k in (0, 1):
                lhsT = mineT[:, k, m * P:(m + 1) * P]
                rhs = mineT[:, k, c0:c0 + W]   # N=512: same f32r mode as main
                nc.tensor.matmul(psd[:, 0:W], lhsT, rhs,
                                 start=(k == 0), stop=(k == 1))
            nc.scalar.activation(psd[:, off:off + P], psd[:, off:off + P],
                                 AF.Exp, scale=1.0 / TEMPERATURE)
            scrd = scr_pool.tile([P, D], F32, tag="scr", name=f"scrd{m}")
            nc.vector.scalar_tensor_tensor(
                out=scrd[:, 0:P], in0=psd[:, off:off + P], scalar=1.0,
                in1=ident_t, op0=MUL, op1=MUL,
                accum_out=stats_sb[:, 4 * mrow + m:4 * mrow + m + 1])

        accs = [acc_pool.tile([P, nslots], F32, tag="acc", name=f"acc{m}")
                for m in range(mrow)]

        # ---- interleaved: column-chunk prologue + that group's matmuls ----
        tiles_per_g = gw // P
        for g in range(ng):
            t0 = g * tiles_per_g
            prologue_chunk([fa_r[t0 + i, :, :] for i in range(tiles_per_g)],
                           allT, t0, None)
            for m in range(mrow):
                psg = ps_pool.tile([P, gw], F32, tag="ps")
                for k in (0, 1):
                    lhsT = mineT[:, k, m * P:(m + 1) * P]
                    for sub in range(nsub):
                        ncol = (g * nsub + sub) * 512
                        nc.tensor.matmul(
                            psg[:, sub * 512:(sub + 1) * 512], lhsT,
                            allT[:, k, ncol:ncol + 512],
                            start=(k == 0), stop=(k == 1))
                for slot, (gg, s, e, _lab) in enumerate(ranges):
                    if gg != g:
                        continue
                    rs, re = s - g * gw, e - g * gw
                    nc.scalar.activation(
                        psg[:, rs:re], psg[:, rs:re], AF.Exp,
                        scale=1.0 / TEMPERATURE,
                        accum_out=accs[m][:, slot:slot + 1])

        # ---- per-row-tile S0/S1 ----
        for m in range(mrow):
            s0 = stats_sb[:, 2 * mrow + m:2 * mrow + m + 1]
            s1 = stats_sb[:, 3 * mrow + m:3 * mrow + m + 1]
            if k0 > 0:
                nc.vector.reduce_sum(s0, accs[m][:, 0:k0], axis=AX)
            else:
                nc.vector.memset(s0, 0.0)
            if k1 > 0:
                nc.vector.reduce_sum(s1, accs[m][:, k0:nslots], axis=AX)
            else:
                nc.vector.memset(s1, 0.0)

        nc.sync.dma_start(stats_d, stats_sb)

    nc.compile()
    return nc


_PROGRAM_CACHE = {}


def _get_program(n0):
    key = (n0, MAT_MODE)
    if key not in _PROGRAM_CACHE:
        _PROGRAM_CACHE[key] = build_program(n0)
    return _PROGRAM_CACHE[key]


def run_device(features, labels, trace=False):
    """Run the Bass kernel on 8 cores.  Returns (per-row device stats dict
    aligned to the label-sorted permutation, permutation order, n0, raw
    BassKernelResults)."""
    B, d = features.shape
    assert d == D and B % NCORES == 0
    bpc = B // NCORES
    mrow = bpc // P

    order = np.argsort(labels, kind="stable")
    n0 = int((labels == 0).sum())
    fp = np.ascontiguousarray(features[order]).astype(np.float32, copy=False)

    nc = _get_program(n0)
    ident = np.eye(P, dtype=np.float32)
    in_maps = [
        {"feat_all": fp,
         "feat_mine": np.ascontiguousarray(fp[c * bpc:(c + 1) * bpc]),
         "ident": ident}
        for c in range(NCORES)
    ]
    res = run_bass_kernel_spmd(nc, in_maps, list(range(NCORES)), trace=trace)

    parts = []
    for c in range(NCORES):
        st = res.results[c]["stats"]          # [128, 5*mrow]
        arr = st.reshape(P, 5, mrow).transpose(1, 2, 0).reshape(5, bpc)
        parts.append(arr)
    full = np.concatenate(parts, axis=1)      # [5, B] in permuted row order
    stats = {"norms2": full[0], "rowsum": full[1], "S0": full[2],
             "S1": full[3], "d": full[4]}
    return stats, order, n0, res


def finalize(stats, order, n0, labels, normal_center, running_sigma, B):
    """Host O(B) finalization mirroring the reference formulas (float64)."""
    labels_p = labels[order]
    nmf = (labels_p == 0)
    amf = (labels_p == 1)
    norms2 = stats["norms2"].astype(np.float64)
    rowsum = stats["rowsum"].astype(np.float64)
    S0 = stats["S0"].astype(np.float64)
    S1 = stats["S1"].astype(np.float64)
    ddiag = stats["d"].astype(np.float64)

    c = np.asarray(normal_center, dtype=np.float64)
    csq = float((c * c).sum())
    if csq != 0.0:
        # general-center path (never hit for this problem's inputs)
        raise NotImplementedError  # replaced by caller passing qc
    dist_sq = norms2  # center == 0
    n_normal = float(nmf.sum())

    with np.errstate(divide="ignore", invalid="ignore"):
        n_el = n_normal * D
        masked_sum = float((rowsum * nmf).sum())
        mean = masked_sum / n_el
        sum_sq_m = float((norms2 * nmf).sum())
        var = (sum_sq_m - 2.0 * mean * masked_sum + mean * mean * n_el) / (n_el - 1.0)
        sigma_new = 0.9 * float(running_sigma) + 0.1 * np.sqrt(var)

        m_adaptive = (MARGIN_BASE + LAMBDA_SIGMA * sigma_new
                      + LAMBDA_RESOLUTION * (1.0 - RESOLUTION_RATIO))
        dist = np.sqrt(np.maximum(dist_sq, 0.0))
        r_center = dist_sq * nmf
        r_margin = np.maximum(m_adaptive - dist, 0.0) * amf

        S_same = np.where(nmf, S0, S1)
        S_diff = np.where(nmf, S1, S0)
        pos_sum = S_same - ddiag
        neg_sum = S_diff
        n1 = B - n0
        cnt_pos = np.where(nmf, n0 - 1, n1 - 1)
        cnt_neg = np.where(nmf, n1, n0)
        has_both = (cnt_pos > 0) & (cnt_neg > 0)
        pos_safe = np.where(has_both, np.maximum(pos_sum, 1e-12), 1.0)
        den_safe = np.where(has_both, pos_sum + neg_sum + 1e-8, 1.0)
        r_con = np.where(has_both, -np.log(pos_safe / den_safe), 0.0)

        raw_total = ALPHA * r_center + BETA * r_margin + GAMMA * r_con
        total = raw_total.mean()
    return np.array(total, dtype=np.float32)


def _finalize_general_center(stats, order, n0, labels, normal_center,
                             running_sigma, B, features):
    """Fallback for a nonzero normal_center (not used for spec inputs)."""
    labels_p = labels[order]
    fp = features[order].astype(np.float64)
    c = np.asarray(normal_center, dtype=np.float64)
    qc = fp @ c
    norms2 = stats["norms2"].astype(np.float64)
    dist_sq = norms2 - 2.0 * qc + float((c * c).sum())
    st2 = dict(stats)
    st2["norms2"] = norms2  # sigma path uses raw norms^2 regardless
    # reuse finalize() with patched dist_sq by inlining:
    nmf = (labels_p == 0)
    amf = (labels_p == 1)
    rowsum = stats["rowsum"].astype(np.float64)
    S0 = stats["S0"].astype(np.float64)
    S1 = stats["S1"].astype(np.float64)
    ddiag = stats["d"].astype(np.float64)
    n_normal = float(nmf.sum())
    with np.errstate(divide="ignore", invalid="ignore"):
        n_el = n_normal * D
        masked_sum = float((rowsum * nmf).sum())
        mean = masked_sum / n_el
        sum_sq_m = float((norms2 * nmf).sum())
        var = (sum_sq_m - 2.0 * mean * masked_sum + mean * mean * n_el) / (n_el - 1.0)
        sigma_new = 0.9 * float(running_sigma) + 0.1 * np.sqrt(var)
        m_adaptive = (MARGIN_BASE + LAMBDA_SIGMA * sigma_new
                      + LAMBDA_RESOLUTION * (1.0 - RESOLUTION_RATIO))
        dist = np.sqrt(np.maximum(dist_sq, 0.0))
        r_center = dist_sq * nmf
        r_margin = np.maximum(m_adaptive - dist, 0.0) * amf
        S_same = np.where(nmf, S0, S1)
        S_diff = np.where(nmf, S1, S0)
        pos_sum = S_same - ddiag
        neg_sum = S_diff
        n1 = B - n0
        cnt_pos = np.where(nmf, n0 - 1, n1 - 1)
        cnt_neg = np.where(nmf, n1, n0)
        has_both = (cnt_pos > 0) & (cnt_neg > 0)
        pos_safe = np.where(has_both, np.maximum(pos_sum, 1e-12), 1.0)
        den_safe = np.where(has_both, pos_sum + neg_sum + 1e-8, 1.0)
        r_con = np.where(has_both, -np.log(pos_safe / den_safe), 0.0)
        total = (ALPHA * r_center + BETA * r_margin + GAMMA * r_con).mean()
    return np.array(total, dtype=np.float32)


def kernel(features, labels, normal_center, running_sigma):
    features = np.asarray(features, dtype=np.float32)
    labels = np.asarray(labels, dtype=np.int32)
    normal_center = np.asarray(normal_center, dtype=np.float32)
    running_sigma = np.float32(np.asarray(running_sigma))
    B = features.shape[0]

    stats, order, n0, _res = run_device(features, labels)
    if float((np.asarray(normal_center, np.float64) ** 2).sum()) != 0.0:
        return _finalize_general_center(stats, order, n0, labels,
                                        normal_center, running_sigma, B,
                                        features)
    return finalize(stats, order, n0, labels, normal_center, running_sigma, B)



# revision 2
# speedup vs baseline: 2.2652x; 2.2652x over previous
"""MACCL loss kernel for Trainium2 (8 NeuronCores, SPMD data-parallel).

Strategy (v2)
-------------
The O(B^2 D) contrastive part dominates (B=8192, D=256).  The host does
the O(B*D) data prep that used to run on-device (and was the pipeline
bottleneck): permute rows label-0-first, compute row norms, quantize the
transposed features to fp8(e4m3) in the [K=128, 2, B] DoubleRow layout.
Each core then only runs the O(B^2) part:

  - 8x 1024-wide fp8 DoubleRow matmuls per 2048-column group: one
    instruction contracts the full K=256 (two k-tiles packed), so the
    PE does 0.5 cyc/row and the whole sim matrix costs ~14us/core.
  - ACT exp(scale_i * psum) in place, scale = r_i/T as a per-partition
    AP (the row normalization folds into the activation for free), with
    accum_out producing per-label-segment row sums (columns are
    label-sorted, segments are contiguous).
  - The diagonal term is reproduced bit-exactly by a DR matmul over the
    core's own (lhsT unscaled, rhs r_j-scaled) fp8 slices -- the same
    host arrays the main loop consumes -- so pos_sum = S_same - d
    cancels exactly on the host.

Per-core output: stats [128, 24] = {S0, S1, exp(diag)} x 8 row tiles.
Host finalizes in f64 (center/margin/sigma/log/mean) exactly mirroring
the reference formulas; norms/rowsums for the center and sigma terms
are host-side f64 (more accurate than the reference's own f32).

The operand quantization error only touches r_con, which is ~0.3% of
the total loss, so fp8 keeps the end-to-end error ~1e-5.
"""

import os
import sys

for _p in ("/root/.axon_site", "/root/.axon_site/_ro/trn_rl_repo",
           "/root/.axon_site/_ro/pypackages", "/opt/trn_rl_repo", "/opt/pypackages"):
    if os.path.isdir(_p) and _p not in sys.path:
        sys.path.append(_p)

import numpy as np
import ml_dtypes
from contextlib import ExitStack

import concourse.bass as bass
import concourse.bacc as bacc
import concourse.tile as tile
from concourse import mybir
from concourse.bass_utils import run_bass_kernel_spmd

F32 = mybir.dt.float32
BF16 = mybir.dt.bfloat16
F8 = mybir.dt.float8e4

P = 128
D = 256
B = 8192
NCORES = 8
BPC = B // NCORES
MROW = BPC // P
GW = 2048
NG = B // GW
TEMPERATURE = 0.07
MARGIN_BASE = 0.5
LAMBDA_SIGMA = 0.3
LAMBDA_RESOLUTION = 0.3
RESOLUTION_RATIO = 224.0 / 900.0
ALPHA, BETA, GAMMA = 1.0, 1.0, 0.5

# "f8dr" = fp8 DoubleRow (1 matmul per chunk, 0.5 cyc/row)
# "bf16" = bf16 with explicit k-loop (fallback)
MODE = os.environ.get("MACCL_MODE", "f8dr")
# moving-operand width per matmul (psum bank = 512 f32)
N_MM = int(os.environ.get("MACCL_N_MM", "512"))


def _segment_ranges(n0, gw=GW):
    """Column ranges per gw-wide group, split at the label boundary n0."""
    ranges = []
    for g in range(B // gw):
        lo, hi = g * gw, (g + 1) * gw
        cuts = sorted({lo, hi, min(max(n0, lo), hi)})
        for s, e in zip(cuts, cuts[1:]):
            if e > s:
                ranges.append((g, s, e, 0 if e <= n0 else 1))
    k0 = sum(1 for r in ranges if r[3] == 0)
    return ranges, k0


def build_program(n0, mode=MODE):
    use_f8 = mode == "f8dr"
    op_dt = F8 if use_f8 else BF16
    perf = mybir.MatmulPerfMode.DoubleRow if use_f8 else None

    ranges, k0 = _segment_ranges(n0)
    nslots = len(ranges)
    k1 = nslots - k0

    AX = mybir.AxisListType.X
    MUL = mybir.AluOpType.mult
    AF = mybir.ActivationFunctionType

    nc = bacc.Bacc("TRN2", target_bir_lowering=False, debug=False,
                   num_devices=NCORES)
    a8_d = nc.dram_tensor("a8", [P, 2, B], op_dt, kind="ExternalInput").ap()
    m8_d = nc.dram_tensor("m8", [P, 2, BPC], op_dt, kind="ExternalInput").ap()
    ms8_d = nc.dram_tensor("ms8", [P, 2, BPC], op_dt, kind="ExternalInput").ap()
    rot_d = nc.dram_tensor("rot", [P, MROW], F32, kind="ExternalInput").ap()
    ident_d = nc.dram_tensor("ident", [P, P], F32, kind="ExternalInput").ap()
    stats_d = nc.dram_tensor("stats", [P, 3 * MROW], F32, kind="ExternalOutput").ap()

    with tile.TileContext(nc) as tc, ExitStack() as ctx:
        singles = ctx.enter_context(tc.tile_pool(name="singles", bufs=1))
        scr_pool = ctx.enter_context(tc.tile_pool(name="scr", bufs=2))
        acc_pool = ctx.enter_context(tc.tile_pool(name="acc", bufs=MROW))
        ps_pool = ctx.enter_context(tc.tile_pool(name="ps", bufs=2, space="PSUM"))

        a8_sb = singles.tile([P, 2, B], op_dt)
        m8_sb = singles.tile([P, 2, BPC], op_dt)
        ms8_sb = singles.tile([P, 2, BPC], op_dt)
        rot_sb = singles.tile([P, MROW], F32)
        ident_t = singles.tile([P, P], F32)
        stats_sb = singles.tile([P, 3 * MROW], F32)

        nc.sync.dma_start(rot_sb, rot_d)
        nc.sync.dma_start(ident_t, ident_d)
        nc.sync.dma_start(m8_sb, m8_d)
        nc.sync.dma_start(ms8_sb, ms8_d)
        # column-group chunks so group-0 matmuls start early
        for g in range(NG):
            nc.sync.dma_start(a8_sb[:, :, g * GW:(g + 1) * GW],
                              a8_d[:, :, g * GW:(g + 1) * GW])

        def mm(out_ap, lhsT, rhs_tile, c0, n):
            if use_f8:
                nc.tensor.matmul(out_ap, lhsT, rhs_tile[:, :, c0:c0 + n],
                                 start=True, stop=True, perf_mode=perf)
            else:
                for k in (0, 1):
                    nc.tensor.matmul(out_ap, lhsT[:, k, :],
                                     rhs_tile[:, k, c0:c0 + n],
                                     start=(k == 0), stop=(k == 1))

        # ---- exact diagonal terms (also warms up the PE early) ----
        for m in range(MROW):
            psd = ps_pool.tile([P, GW], F32, tag="ps", name=f"psd{m}")
            lhsT = m8_sb[:, :, m * P:(m + 1) * P]
            for s in range(BPC // N_MM):
                mm(psd[:, s * N_MM:(s + 1) * N_MM], lhsT, ms8_sb, s * N_MM, N_MM)
            off = m * P
            nc.scalar.activation(psd[:, off:off + P], psd[:, off:off + P],
                                 AF.Exp, scale=rot_sb[:, m:m + 1])
            scrd = scr_pool.tile([P, P], F32, tag="scr", name=f"scrd{m}")
            nc.vector.scalar_tensor_tensor(
                out=scrd, in0=psd[:, off:off + P], scalar=1.0,
                in1=ident_t, op0=MUL, op1=MUL,
                accum_out=stats_sb[:, 2 * MROW + m:2 * MROW + m + 1])

        accs = [acc_pool.tile([P, nslots], F32, tag="acc", name=f"acc{m}")
                for m in range(MROW)]

        # ---- main loop: per column group, per own row tile ----
        for g in range(NG):
            for m in range(MROW):
                psg = ps_pool.tile([P, GW], F32, tag="ps")
                lhsT = m8_sb[:, :, m * P:(m + 1) * P]
                for s in range(GW // N_MM):
                    mm(psg[:, s * N_MM:(s + 1) * N_MM], lhsT, a8_sb,
                       g * GW + s * N_MM, N_MM)
                for slot, (gg, s, e, _lab) in enumerate(ranges):
                    if gg != g:
                        continue
                    rs, re = s - g * GW, e - g * GW
                    nc.scalar.activation(
                        psg[:, rs:re], psg[:, rs:re], AF.Exp,
                        scale=rot_sb[:, m:m + 1],
                        accum_out=accs[m][:, slot:slot + 1])

        # ---- per-row-tile S0/S1 ----
        for m in range(MROW):
            s0 = stats_sb[:, m:m + 1]
            s1 = stats_sb[:, MROW + m:MROW + m + 1]
            if k0 > 0:
                nc.vector.reduce_sum(s0, accs[m][:, 0:k0], axis=AX)
            else:
                nc.vector.memset(s0, 0.0)
            if k1 > 0:
                nc.vector.reduce_sum(s1, accs[m][:, k0:nslots], axis=AX)
            else:
                nc.vector.memset(s1, 0.0)

        nc.sync.dma_start(stats_d, stats_sb)

    nc.compile()
    return nc


_PROGRAM_CACHE = {}


def _get_program(n0):
    key = (n0, MODE, N_MM)
    if key not in _PROGRAM_CACHE:
        _PROGRAM_CACHE[key] = build_program(n0)
    return _PROGRAM_CACHE[key]


def run_device(features, labels, trace=False):
    """Host prep + 8-core device run.  Returns (stats dict aligned to the
    label-sorted permutation, permutation order, n0, raw results)."""
    Bq, d = features.shape
    assert d == D and Bq == B

    order = np.argsort(labels, kind="stable")
    n0 = int((labels == 0).sum())
    fp = np.ascontiguousarray(features[order]).astype(np.float32, copy=False)

    # host-side O(B*D) prep
    fp64 = fp.astype(np.float64)
    norms2 = (fp64 * fp64).sum(axis=1)                  # [B]
    rowsum = fp64.sum(axis=1)                           # [B]
    r = 1.0 / np.maximum(np.sqrt(norms2), 1e-12)        # [B]
    r32 = r.astype(np.float32)

    op_np = ml_dtypes.float8_e4m3 if MODE == "f8dr" else ml_dtypes.bfloat16
    # [K=128, 2, B] DoubleRow layout: D index = ktile*128 + partition
    fT = np.ascontiguousarray(fp.T).reshape(2, P, B).transpose(1, 0, 2)
    m8_full = np.ascontiguousarray(fT).astype(op_np)                 # unscaled
    a8 = np.ascontiguousarray(fT * r32[None, None, :]).astype(op_np)  # scaled

    rot_full = (r32 / np.float32(TEMPERATURE)).astype(np.float32)
    ident = np.eye(P, dtype=np.float32)

    nc = _get_program(n0)
    in_maps = []
    for c in range(NCORES):
        sl = slice(c * BPC, (c + 1) * BPC)
        in_maps.append({
            "a8": a8,
            "m8": np.ascontiguousarray(m8_full[:, :, sl]),
            "ms8": np.ascontiguousarray(a8[:, :, sl]),
            "rot": np.ascontiguousarray(
                rot_full[sl].reshape(MROW, P).T),
            "ident": ident,
        })
    res = run_bass_kernel_spmd(nc, in_maps, list(range(NCORES)), trace=trace)

    parts = []
    for c in range(NCORES):
        st = res.results[c]["stats"]          # [128, 3*MROW]
        arr = st.reshape(P, 3, MROW).transpose(1, 2, 0).reshape(3, BPC)
        parts.append(arr)
    full = np.concatenate(parts, axis=1)      # [3, B] in permuted row order
    stats = {"norms2": norms2, "rowsum": rowsum,
             "S0": full[0], "S1": full[1], "d": full[2]}
    return stats, order, n0, res


def finalize(stats, order, n0, labels, normal_center, running_sigma, B):
    """Host O(B) finalization mirroring the reference formulas (float64)."""
    labels_p = labels[order]
    nmf = (labels_p == 0)
    amf = (labels_p == 1)
    norms2 = stats["norms2"].astype(np.float64)
    rowsum = stats["rowsum"].astype(np.float64)
    S0 = stats["S0"].astype(np.float64)
    S1 = stats["S1"].astype(np.float64)
    ddiag = stats["d"].astype(np.float64)

    c = np.asarray(normal_center, dtype=np.float64)
    csq = float((c * c).sum())
    if csq != 0.0:
        raise NotImplementedError  # caller routes to the general-center path
    dist_sq = norms2  # center == 0
    n_normal = float(nmf.sum())

    with np.errstate(divide="ignore", invalid="ignore"):
        n_el = n_normal * D
        masked_sum = float((rowsum * nmf).sum())
        mean = masked_sum / n_el
        sum_sq_m = float((norms2 * nmf).sum())
        var = (sum_sq_m - 2.0 * mean * masked_sum + mean * mean * n_el) / (n_el - 1.0)
        sigma_new = 0.9 * float(running_sigma) + 0.1 * np.sqrt(var)

        m_adaptive = (MARGIN_BASE + LAMBDA_SIGMA * sigma_new
                      + LAMBDA_RESOLUTION * (1.0 - RESOLUTION_RATIO))
        dist = np.sqrt(np.maximum(dist_sq, 0.0))
        r_center = dist_sq * nmf
        r_margin = np.maximum(m_adaptive - dist, 0.0) * amf

        S_same = np.where(nmf, S0, S1)
        S_diff = np.where(nmf, S1, S0)
        pos_sum = S_same - ddiag
        neg_sum = S_diff
        n1 = B - n0
        cnt_pos = np.where(nmf, n0 - 1, n1 - 1)
        cnt_neg = np.where(nmf, n1, n0)
        has_both = (cnt_pos > 0) & (cnt_neg > 0)
        pos_safe = np.where(has_both, np.maximum(pos_sum, 1e-12), 1.0)
        den_safe = np.where(has_both, pos_sum + neg_sum + 1e-8, 1.0)
        r_con = np.where(has_both, -np.log(pos_safe / den_safe), 0.0)

        raw_total = ALPHA * r_center + BETA * r_margin + GAMMA * r_con
        total = raw_total.mean()
    return np.array(total, dtype=np.float32)


def _finalize_general_center(stats, order, n0, labels, normal_center,
                             running_sigma, B, features):
    """Fallback for a nonzero normal_center (not hit for spec inputs)."""
    labels_p = labels[order]
    fp = features[order].astype(np.float64)
    c = np.asarray(normal_center, dtype=np.float64)
    qc = fp @ c
    norms2 = stats["norms2"].astype(np.float64)
    dist_sq = norms2 - 2.0 * qc + float((c * c).sum())
    nmf = (labels_p == 0)
    amf = (labels_p == 1)
    rowsum = stats["rowsum"].astype(np.float64)
    S0 = stats["S0"].astype(np.float64)
    S1 = stats["S1"].astype(np.float64)
    ddiag = stats["d"].astype(np.float64)
    n_normal = float(nmf.sum())
    with np.errstate(divide="ignore", invalid="ignore"):
        n_el = n_normal * D
        masked_sum = float((rowsum * nmf).sum())
        mean = masked_sum / n_el
        sum_sq_m = float((norms2 * nmf).sum())
        var = (sum_sq_m - 2.0 * mean * masked_sum + mean * mean * n_el) / (n_el - 1.0)
        sigma_new = 0.9 * float(running_sigma) + 0.1 * np.sqrt(var)
        m_adaptive = (MARGIN_BASE + LAMBDA_SIGMA * sigma_new
                      + LAMBDA_RESOLUTION * (1.0 - RESOLUTION_RATIO))
        dist = np.sqrt(np.maximum(dist_sq, 0.0))
        r_center = dist_sq * nmf
        r_margin = np.maximum(m_adaptive - dist, 0.0) * amf
        S_same = np.where(nmf, S0, S1)
        S_diff = np.where(nmf, S1, S0)
        pos_sum = S_same - ddiag
        neg_sum = S_diff
        n1 = B - n0
        cnt_pos = np.where(nmf, n0 - 1, n1 - 1)
        cnt_neg = np.where(nmf, n1, n0)
        has_both = (cnt_pos > 0) & (cnt_neg > 0)
        pos_safe = np.where(has_both, np.maximum(pos_sum, 1e-12), 1.0)
        den_safe = np.where(has_both, pos_sum + neg_sum + 1e-8, 1.0)
        r_con = np.where(has_both, -np.log(pos_safe / den_safe), 0.0)
        total = (ALPHA * r_center + BETA * r_margin + GAMMA * r_con).mean()
    return np.array(total, dtype=np.float32)


def kernel(features, labels, normal_center, running_sigma):
    features = np.asarray(features, dtype=np.float32)
    labels = np.asarray(labels, dtype=np.int32)
    normal_center = np.asarray(normal_center, dtype=np.float32)
    running_sigma = np.float32(np.asarray(running_sigma))
    Bq = features.shape[0]

    stats, order, n0, _res = run_device(features, labels)
    if float((np.asarray(normal_center, np.float64) ** 2).sum()) != 0.0:
        return _finalize_general_center(stats, order, n0, labels,
                                        normal_center, running_sigma, Bq,
                                        features)
    return finalize(stats, order, n0, labels, normal_center, running_sigma, Bq)
